# revision 22
# baseline (speedup 1.0000x reference)
"""M3GNet interaction kernel for 8 Trainium2 NeuronCores.

Sharding: edges (640000) and triplets (1000000) are split 8 ways
(graph/data parallel, per the sharding hint); weight matrices are
replicated. Each core runs the dense per-edge radial MLP and the
per-triplet angular MLP (first layer + shifted-softplus) on device;
per-node segment sums are combined after gathering the shards.

Device-kernel layout tricks:
 - Pairs of 512-element slices are stacked on partition halves
   (block-diagonal weights), so every matmul / activation runs with
   all 128 partitions instead of 64.
 - Matmuls run in bf16 (edge path) / fp32r (triplet path, N=512 so
   full rate) instead of fp32 (which costs 4 cycles/column).
 - softplus = Ln(1 + Exp(x)) with the +1 folded into the Ln bias; both
   funcs live in one activation table (natural_log_exp_and_others) so
   no ACT_TABLE_LOAD thrash.  The -log2 shift is folded into a bias
   column (edge path) / host-side count correction (triplet path).
"""
import numpy as np

import concourse.bacc as bacc
import concourse.bass as bass
import concourse.mybir as mybir
from concourse.tile import TileContext
from concourse import bass_utils
import concourse.hw_specs as hw_specs

N_NODES = 20000
N_EDGES = 640000
N_TRIP = 1000000
C = 128
E = 64
CUTOFF = 5.0
LOG2 = float(np.log(2.0))
NCORES = 8
EPC = N_EDGES // NCORES      # 80000 edges per core
TPC = N_TRIP // NCORES       # 125000 triplets per core

EPAD = 81920                 # edges padded: 80 pairs of (512+512)
TPAD = 125952                # triplets padded: 123 pairs
ECOLS = EPAD // 2            # 40960 packed columns (two edges/col)
TCOLS = TPAD // 2            # 62976 packed columns

ECH = 4096                   # edge packed-cols per chunk (10 chunks)
TCH = 8192                   # triplet packed-cols per chunk (8 chunks)

GAMMA = 1.0 / (2.0 * (CUTOFF / E) ** 2)

_CACHED = {}


def _patch_act_tables():
    """Restrict activation-table choice to the single table that holds
    Exp+Ln (+Square/Copy), so the compiler stops alternating table
    loads between Exp and Ln (which cost ~675us in the fp32 baseline).
    Table list order (= act_func_set_id) is preserved."""
    if _CACHED.get('act_patched'):
        return
    orig = hw_specs.get_activation_tables

    def patched(arch):
        return {k: (v if k == 'natural_log_exp_and_others' else set())
                for k, v in orig(arch).items()}

    bacc.get_activation_tables = patched
    _CACHED['act_patched'] = True


def _build():
    if 'nc' in _CACHED:
        return _CACHED['nc']
    _patch_act_tables()
    nc = bacc.Bacc('TRN2', target_bir_lowering=False, debug=False)
    f32 = mybir.dt.float32
    bf = mybir.dt.bfloat16
    f32r = mybir.dt.float32r

    rbe = nc.dram_tensor('rbe', [128, ECOLS], bf, kind='ExternalInput')
    tbf = nc.dram_tensor('tbf', [6, TCOLS], f32r, kind='ExternalInput')
    # wcat = [w1blk | w2dup] bf16; fcat = [w3blk_padded | corr] fp32(r)
    wcat = nc.dram_tensor('wcat', [128, 256], bf, kind='ExternalInput')
    fcat = nc.dram_tensor('fcat', [128, 129], f32r, kind='ExternalInput')

    moT = nc.dram_tensor('moT', [C, EPAD], bf, kind='ExternalOutput')
    uT = nc.dram_tensor('uT', [128, TCOLS], bf, kind='ExternalOutput')

    AF = mybir.ActivationFunctionType
    with TileContext(nc) as tc:
        with (
            tc.tile_pool(name='wpool', bufs=1) as wp,
            tc.tile_pool(name='rbe_in', bufs=2) as rin,
            tc.tile_pool(name='tbf_in', bufs=2) as tin,
            tc.tile_pool(name='expbuf', bufs=2) as eb,
            tc.tile_pool(name='spbuf', bufs=2) as sb,
            tc.tile_pool(name='outbuf', bufs=2) as ob,
            tc.tile_pool(name='psA', bufs=2, space='PSUM') as psA,
            tc.tile_pool(name='psB', bufs=2, space='PSUM') as psB,
        ):
            wct = wp.tile([128, 256], bf, tag='wc')
            nc.sync.dma_start(wct[:], wcat[:])
            fct = wp.tile([128, 129], f32r, tag='fc')
            nc.sync.dma_start(fct[:], fcat[:])
            w1t = wct[:, 0:128]
            w2t = wct[:, 128:256]
            w3t = fct[0:6, 0:128]
            corr_t = fct[:, 128:129].bitcast(f32)

            # ---- edge chunk: packed col j holds edges (1024p + q) [top]
            # and (1024p + 512 + q) [bottom] where j = 512p + q.
            def edge_chunk(c0):
                cw = min(ECH, ECOLS - c0)
                rt = rin.tile([128, ECH], bf, tag='rbe')
                nc.sync.dma_start(rt[:, :cw], rbe[:, c0:c0 + cw])
                et = eb.tile([128, ECH], bf, tag='e16')
                for q0 in range(0, cw, 1024):
                    qw = min(1024, cw - q0)
                    pp = psA.tile([128, 1024], f32, tag='pp')
                    for s0 in range(0, qw, 512):
                        nc.tensor.matmul(pp[:, s0:s0 + 512], w1t[:],
                                         rt[:, q0 + s0:q0 + s0 + 512])
                    nc.scalar.activation(et[:, q0:q0 + qw], pp[:, :qw], AF.Exp)
                st = sb.tile([128, ECH], bf, tag='sp')
                nc.scalar.activation(st[:, :cw], et[:, :cw], AF.Ln, bias=1.0)
                mt = ob.tile([128, 2 * ECH], bf, tag='mo')
                for p in range(cw // 512):
                    j0 = p * 512
                    pq = psB.tile([128, 1024], f32, tag='pq')
                    nc.tensor.matmul(pq[:, :512], w2t[0:64, :],
                                     st[0:64, j0:j0 + 512])
                    nc.tensor.matmul(pq[:, 512:], w2t[64:128, :],
                                     st[64:128, j0:j0 + 512])
                    if p % 8 < 3:
                        # ACT has slack in the edge phase; DVE is the
                        # edge-phase bottleneck — split evictions 5:3.
                        nc.scalar.activation(mt[:, 2 * j0:2 * j0 + 1024],
                                             pq[:], AF.Identity,
                                             bias=corr_t[:])
                    else:
                        nc.vector.tensor_scalar(mt[:, 2 * j0:2 * j0 + 1024],
                                                pq[:], corr_t[:], None,
                                                mybir.AluOpType.add)
                nc.sync.dma_start(moT[:, 2 * c0:2 * c0 + 2 * cw],
                                  mt[:, :2 * cw])

            # ---- triplet chunk: u = ln(1 + exp(tbf @ W3b1)) ----
            def trip_chunk(c0):
                cw = min(TCH, TCOLS - c0)
                tt = tin.tile([6, TCH], f32r, tag='tb')
                nc.sync.dma_start(tt[:, :cw], tbf[:, c0:c0 + cw])
                ut = ob.tile([128, TCH], bf, tag='u16')
                for g0 in range(0, cw, 4096):
                    gw = min(4096, cw - g0)
                    e3 = eb.tile([128, 4096], bf, tag='e3')
                    for q0 in range(g0, g0 + gw, 1024):
                        qw = min(1024, g0 + gw - q0)
                        pp = psA.tile([128, 1024], f32, tag='pp')
                        for s0 in range(0, qw, 512):
                            nc.tensor.matmul(
                                pp[:, s0:s0 + 512],
                                w3t[:], tt[:, q0 + s0:q0 + s0 + 512])
                        nc.scalar.activation(e3[:, q0 - g0:q0 - g0 + qw],
                                             pp[:, :qw], AF.Exp)
                    nc.scalar.activation(ut[:, g0:g0 + gw], e3[:, :gw],
                                         AF.Ln, bias=1.0)
                    # per-half DMA keeps the kernel-tail transfer small
                    nc.sync.dma_start(uT[:, c0 + g0:c0 + g0 + gw],
                                      ut[:, g0:g0 + gw])

            for c0 in range(0, ECOLS, ECH):
                edge_chunk(c0)
            for c0 in range(0, TCOLS, TCH):
                trip_chunk(c0)

    nc.compile()
    _CACHED['nc'] = nc
    return nc


def _segsum(vals, idx, nseg):
    """f64-accurate segment sum via sort + cumsum (duplicate-safe)."""
    order = np.argsort(idx, kind='stable')
    sidx = idx[order]
    cs = np.cumsum(vals[order].astype(np.float64), axis=0)
    csz = np.vstack([np.zeros((1, vals.shape[1])), cs])
    starts = np.searchsorted(sidx, np.arange(nseg), side='left')
    ends = np.searchsorted(sidx, np.arange(nseg), side='right')
    return (csz[ends] - csz[starts]).astype(np.float32)


def _pack_pairs_edges(x):
    """[EPAD, 64] -> [128, ECOLS]: col 512p+q holds rows 1024p+q (top
    64 partitions) and 1024p+512+q (bottom 64)."""
    return np.ascontiguousarray(
        x.reshape(-1, 2, 512, 64).transpose(1, 3, 0, 2).reshape(128, -1))


def _pack_pairs_tbf(x):
    """[3, TPAD] -> [6, TCOLS]: col 512p+q holds triplet 1024p+q
    (rows 0-2) and 1024p+512+q (rows 3-5)."""
    return np.ascontiguousarray(
        x.reshape(3, -1, 2, 512).transpose(2, 0, 1, 3).reshape(6, -1))


def _unpack_pairs_u(uT):
    """[128, TCOLS] -> [TPAD, 64] (inverse of the pair packing)."""
    return uT.reshape(2, 64, -1, 512).transpose(2, 0, 3, 1).reshape(-1, 64)


def kernel(features, neighbour_distances, neighbour_list, triplet_idxs,
           angles, r_ij, r_ik, W_pre, W2b1, W2b2, W3b1, W3b2, W_post):
    nc = _build()
    bf16 = mybir.dt.np(mybir.dt.bfloat16)

    d = np.asarray(neighbour_distances, np.float32)
    env = (0.5 * (1.0 + np.cos(np.pi * d / CUTOFF))
           * (d < CUTOFF)).astype(np.float32)
    centers = np.linspace(0.0, CUTOFF, E, dtype=np.float32)
    rbe_full = (np.exp(-GAMMA * (d[:, None] - centers[None, :]) ** 2)
                * env[:, None]).astype(np.float32)          # [Ne, 64]
    tbf_full = np.stack([np.asarray(r_ij, np.float32),
                         np.asarray(r_ik, np.float32),
                         np.cos(np.asarray(angles, np.float32))], axis=0)

    W2b1 = np.asarray(W2b1, np.float32)
    W2b2 = np.asarray(W2b2, np.float32)
    W3b1 = np.asarray(W3b1, np.float32)
    wcat = np.zeros((128, 256), np.float32)
    wcat[:64, :64] = W2b1            # w1blk block-diagonal
    wcat[64:, 64:128] = W2b1
    wcat[:64, 128:] = W2b2           # w2dup (both halves)
    wcat[64:, 128:] = W2b2
    fcat = np.zeros((128, 129), np.float32)
    fcat[0:3, 0:64] = W3b1           # w3blk block-diagonal
    fcat[3:6, 64:128] = W3b1
    fcat[:, 128] = -LOG2 * W2b2.sum(axis=0)   # corr column

    shared = {
        'wcat': wcat.astype(bf16),
        'fcat': np.ascontiguousarray(fcat),
    }
    in_maps = []
    for k in range(NCORES):
        ec = np.zeros((EPAD, E), np.float32)
        ec[:EPC] = rbe_full[k * EPC:(k + 1) * EPC]
        tc_ = np.zeros((3, TPAD), np.float32)
        tc_[:, :TPC] = tbf_full[:, k * TPC:(k + 1) * TPC]
        in_maps.append(dict(shared,
                            rbe=_pack_pairs_edges(ec).astype(bf16),
                            tbf=_pack_pairs_tbf(tc_)))

    res = bass_utils.run_bass_kernel_spmd(nc, in_maps,
                                          core_ids=list(range(NCORES)))
    kernel.last_results = res

    m = np.concatenate(
        [r['moT'][:, :EPC].astype(np.float32).T for r in res.results],
        axis=0)                                            # [Ne, C]
    u = np.concatenate(
        [_unpack_pairs_u(r['uT'].astype(np.float32))[:TPC]
         for r in res.results], axis=0)                    # [Nt, E]

    h = np.asarray(features, np.float32) @ np.asarray(W_pre, np.float32)
    nl0 = np.asarray(neighbour_list)[0]
    nl1 = np.asarray(neighbour_list)[1]
    t1 = np.asarray(triplet_idxs)[:, 1]

    two_body = h[nl1] * m
    agg = _segsum(two_body, nl0, N_NODES)

    U3 = _segsum(u, t1, N_NODES)
    U3 -= LOG2 * np.bincount(t1, minlength=N_NODES)[:, None]
    em = h[:N_NODES] * (U3 @ np.asarray(W3b2, np.float32))
    agg += _segsum(em, nl0[:N_NODES], N_NODES)

    return (agg @ np.asarray(W_post, np.float32)).astype(np.float32)


# revision 23
# speedup vs baseline: 1.0848x; 1.0848x over previous
"""M3GNet interaction kernel for 8 Trainium2 NeuronCores.

Sharding: edges (640000) and triplets (1000000) are split 8 ways
(graph/data parallel, per the sharding hint); weight matrices are
replicated. Each core runs the dense per-edge radial MLP and the
per-triplet angular MLP (first layer + shifted-softplus) on device;
per-node segment sums are combined after gathering the shards.

Device-kernel layout tricks:
 - Pairs of 512-element slices are stacked on partition halves
   (block-diagonal weights), so every matmul / activation runs with
   all 128 partitions instead of 64.
 - Matmuls run in bf16 (edge path) / fp32r (triplet path, N=512 so
   full rate) instead of fp32 (which costs 4 cycles/column).
 - softplus = Ln(1 + Exp(x)) with the +1 folded into the Ln bias; both
   funcs live in one activation table (natural_log_exp_and_others) so
   no ACT_TABLE_LOAD thrash.  The -log2 shift is folded into a bias
   column (edge path) / host-side count correction (triplet path).
"""
import numpy as np

import concourse.bacc as bacc
import concourse.bass as bass
import concourse.mybir as mybir
from concourse.tile import TileContext
from concourse import bass_utils
import concourse.hw_specs as hw_specs

N_NODES = 20000
N_EDGES = 640000
N_TRIP = 1000000
C = 128
E = 64
CUTOFF = 5.0
LOG2 = float(np.log(2.0))
NCORES = 8
EPC = N_EDGES // NCORES      # 80000 edges per core
TPC = N_TRIP // NCORES       # 125000 triplets per core

EPAD = 81920                 # edges padded: 80 pairs of (512+512)
TPAD = 125952                # triplets padded: 123 pairs
ECOLS = EPAD // 2            # 40960 packed columns (two edges/col)
TCOLS = TPAD // 2            # 62976 packed columns

ECH = 4096                   # edge packed-cols per chunk (10 chunks)
TCH = 8192                   # triplet packed-cols per chunk (8 chunks)

GAMMA = 1.0 / (2.0 * (CUTOFF / E) ** 2)

_CACHED = {}


def _patch_act_tables():
    """Restrict activation-table choice to the single table that holds
    Exp+Ln (+Square/Copy), so the compiler stops alternating table
    loads between Exp and Ln (which cost ~675us in the fp32 baseline).
    Table list order (= act_func_set_id) is preserved."""
    if _CACHED.get('act_patched'):
        return
    orig = hw_specs.get_activation_tables

    def patched(arch):
        return {k: (v if k == 'natural_log_exp_and_others' else set())
                for k, v in orig(arch).items()}

    bacc.get_activation_tables = patched
    _CACHED['act_patched'] = True


def _build():
    if 'nc' in _CACHED:
        return _CACHED['nc']
    _patch_act_tables()
    nc = bacc.Bacc('TRN2', target_bir_lowering=False, debug=False)
    f32 = mybir.dt.float32
    bf = mybir.dt.bfloat16
    f32r = mybir.dt.float32r

    rbe = nc.dram_tensor('rbe', [128, ECOLS], bf, kind='ExternalInput')
    tbf = nc.dram_tensor('tbf', [6, TCOLS], f32r, kind='ExternalInput')
    # wcat = [w1blk | w2dup] bf16; fcat = [w3blk_padded | corr] fp32(r)
    wcat = nc.dram_tensor('wcat', [128, 256], bf, kind='ExternalInput')
    fcat = nc.dram_tensor('fcat', [128, 129], f32r, kind='ExternalInput')

    moT = nc.dram_tensor('moT', [C, EPAD], bf, kind='ExternalOutput')
    uT = nc.dram_tensor('uT', [128, TCOLS], bf, kind='ExternalOutput')

    AF = mybir.ActivationFunctionType
    with TileContext(nc) as tc:
        with (
            tc.tile_pool(name='wpool', bufs=1) as wp,
            tc.tile_pool(name='rbe_in', bufs=2) as rin,
            tc.tile_pool(name='tbf_in', bufs=2) as tin,
            tc.tile_pool(name='expbuf', bufs=2) as eb,
            tc.tile_pool(name='spbuf', bufs=2) as sb,
            tc.tile_pool(name='outbuf', bufs=2) as ob,
            tc.tile_pool(name='psA', bufs=2, space='PSUM') as psA,
            tc.tile_pool(name='psB', bufs=2, space='PSUM') as psB,
        ):
            wct = wp.tile([128, 256], bf, tag='wc')
            nc.sync.dma_start(wct[:], wcat[:])
            fct = wp.tile([128, 129], f32r, tag='fc')
            nc.sync.dma_start(fct[:], fcat[:])
            w1t = wct[:, 0:128]
            w2t = wct[:, 128:256]
            w3t = fct[0:6, 0:128]
            corr_t = fct[:, 128:129].bitcast(f32)

            # ---- edge chunk: packed col j holds edges (1024p + q) [top]
            # and (1024p + 512 + q) [bottom] where j = 512p + q.
            def edge_chunk(c0):
                cw = min(ECH, ECOLS - c0)
                rt = rin.tile([128, ECH], bf, tag='rbe')
                nc.sync.dma_start(rt[:, :cw], rbe[:, c0:c0 + cw])
                et = eb.tile([128, ECH], bf, tag='e16')
                for q0 in range(0, cw, 1024):
                    qw = min(1024, cw - q0)
                    pp = psA.tile([128, 1024], f32, tag='pp')
                    for s0 in range(0, qw, 512):
                        nc.tensor.matmul(pp[:, s0:s0 + 512], w1t[:],
                                         rt[:, q0 + s0:q0 + s0 + 512])
                    nc.scalar.activation(et[:, q0:q0 + qw], pp[:, :qw], AF.Exp)
                st = sb.tile([128, ECH], bf, tag='sp')
                nc.scalar.activation(st[:, :cw], et[:, :cw], AF.Ln, bias=1.0)
                mt = ob.tile([128, 2 * ECH], bf, tag='mo')
                for p in range(cw // 512):
                    j0 = p * 512
                    pq = psB.tile([128, 1024], f32, tag='pq')
                    nc.tensor.matmul(pq[:, :512], w2t[0:64, :],
                                     st[0:64, j0:j0 + 512])
                    nc.tensor.matmul(pq[:, 512:], w2t[64:128, :],
                                     st[64:128, j0:j0 + 512])
                    nc.vector.tensor_scalar(mt[:, 2 * j0:2 * j0 + 1024],
                                            pq[:], corr_t[:], None,
                                            mybir.AluOpType.add)
                nc.sync.dma_start(moT[:, 2 * c0:2 * c0 + 2 * cw],
                                  mt[:, :2 * cw])

            # ---- triplet chunk: u = ln(1 + exp(tbf @ W3b1)) ----
            def trip_chunk(c0):
                cw = min(TCH, TCOLS - c0)
                tt = tin.tile([6, TCH], f32r, tag='tb')
                nc.sync.dma_start(tt[:, :cw], tbf[:, c0:c0 + cw])
                ut = ob.tile([128, TCH], bf, tag='u16')
                for g0 in range(0, cw, 4096):
                    gw = min(4096, cw - g0)
                    e3 = eb.tile([128, 4096], bf, tag='e3')
                    for q0 in range(g0, g0 + gw, 1024):
                        qw = min(1024, g0 + gw - q0)
                        pp = psA.tile([128, 1024], f32, tag='pp')
                        for s0 in range(0, qw, 512):
                            nc.tensor.matmul(
                                pp[:, s0:s0 + 512],
                                w3t[:], tt[:, q0 + s0:q0 + s0 + 512])
                        nc.scalar.activation(e3[:, q0 - g0:q0 - g0 + qw],
                                             pp[:, :qw], AF.Exp)
                    nc.scalar.activation(ut[:, g0:g0 + gw], e3[:, :gw],
                                         AF.Ln, bias=1.0)
                    # per-half DMA keeps the kernel-tail transfer small
                    nc.sync.dma_start(uT[:, c0 + g0:c0 + g0 + gw],
                                      ut[:, g0:g0 + gw])

            for c0 in range(0, ECOLS, ECH):
                edge_chunk(c0)
            for c0 in range(0, TCOLS, TCH):
                trip_chunk(c0)

    nc.compile()
    _CACHED['nc'] = nc
    return nc


def _segsum(vals, idx, nseg):
    """f64-accurate segment sum via sort + cumsum (duplicate-safe)."""
    order = np.argsort(idx, kind='stable')
    sidx = idx[order]
    cs = np.cumsum(vals[order].astype(np.float64), axis=0)
    csz = np.vstack([np.zeros((1, vals.shape[1])), cs])
    starts = np.searchsorted(sidx, np.arange(nseg), side='left')
    ends = np.searchsorted(sidx, np.arange(nseg), side='right')
    return (csz[ends] - csz[starts]).astype(np.float32)


def _pack_pairs_edges(x):
    """[EPAD, 64] -> [128, ECOLS]: col 512p+q holds rows 1024p+q (top
    64 partitions) and 1024p+512+q (bottom 64)."""
    return np.ascontiguousarray(
        x.reshape(-1, 2, 512, 64).transpose(1, 3, 0, 2).reshape(128, -1))


def _pack_pairs_tbf(x):
    """[3, TPAD] -> [6, TCOLS]: col 512p+q holds triplet 1024p+q
    (rows 0-2) and 1024p+512+q (rows 3-5)."""
    return np.ascontiguousarray(
        x.reshape(3, -1, 2, 512).transpose(2, 0, 1, 3).reshape(6, -1))


def _unpack_pairs_u(uT):
    """[128, TCOLS] -> [TPAD, 64] (inverse of the pair packing)."""
    return uT.reshape(2, 64, -1, 512).transpose(2, 0, 3, 1).reshape(-1, 64)


def kernel(features, neighbour_distances, neighbour_list, triplet_idxs,
           angles, r_ij, r_ik, W_pre, W2b1, W2b2, W3b1, W3b2, W_post):
    nc = _build()
    bf16 = mybir.dt.np(mybir.dt.bfloat16)

    d = np.asarray(neighbour_distances, np.float32)
    env = (0.5 * (1.0 + np.cos(np.pi * d / CUTOFF))
           * (d < CUTOFF)).astype(np.float32)
    centers = np.linspace(0.0, CUTOFF, E, dtype=np.float32)
    rbe_full = (np.exp(-GAMMA * (d[:, None] - centers[None, :]) ** 2)
                * env[:, None]).astype(np.float32)          # [Ne, 64]
    tbf_full = np.stack([np.asarray(r_ij, np.float32),
                         np.asarray(r_ik, np.float32),
                         np.cos(np.asarray(angles, np.float32))], axis=0)

    W2b1 = np.asarray(W2b1, np.float32)
    W2b2 = np.asarray(W2b2, np.float32)
    W3b1 = np.asarray(W3b1, np.float32)
    wcat = np.zeros((128, 256), np.float32)
    wcat[:64, :64] = W2b1            # w1blk block-diagonal
    wcat[64:, 64:128] = W2b1
    wcat[:64, 128:] = W2b2           # w2dup (both halves)
    wcat[64:, 128:] = W2b2
    fcat = np.zeros((128, 129), np.float32)
    fcat[0:3, 0:64] = W3b1           # w3blk block-diagonal
    fcat[3:6, 64:128] = W3b1
    fcat[:, 128] = -LOG2 * W2b2.sum(axis=0)   # corr column

    shared = {
        'wcat': wcat.astype(bf16),
        'fcat': np.ascontiguousarray(fcat),
    }
    in_maps = []
    for k in range(NCORES):
        ec = np.zeros((EPAD, E), np.float32)
        ec[:EPC] = rbe_full[k * EPC:(k + 1) * EPC]
        tc_ = np.zeros((3, TPAD), np.float32)
        tc_[:, :TPC] = tbf_full[:, k * TPC:(k + 1) * TPC]
        in_maps.append(dict(shared,
                            rbe=_pack_pairs_edges(ec).astype(bf16),
                            tbf=_pack_pairs_tbf(tc_)))

    res = bass_utils.run_bass_kernel_spmd(nc, in_maps,
                                          core_ids=list(range(NCORES)))
    kernel.last_results = res

    m = np.concatenate(
        [r['moT'][:, :EPC].astype(np.float32).T for r in res.results],
        axis=0)                                            # [Ne, C]
    u = np.concatenate(
        [_unpack_pairs_u(r['uT'].astype(np.float32))[:TPC]
         for r in res.results], axis=0)                    # [Nt, E]

    h = np.asarray(features, np.float32) @ np.asarray(W_pre, np.float32)
    nl0 = np.asarray(neighbour_list)[0]
    nl1 = np.asarray(neighbour_list)[1]
    t1 = np.asarray(triplet_idxs)[:, 1]

    two_body = h[nl1] * m
    agg = _segsum(two_body, nl0, N_NODES)

    U3 = _segsum(u, t1, N_NODES)
    U3 -= LOG2 * np.bincount(t1, minlength=N_NODES)[:, None]
    em = h[:N_NODES] * (U3 @ np.asarray(W3b2, np.float32))
    agg += _segsum(em, nl0[:N_NODES], N_NODES)

    return (agg @ np.asarray(W_post, np.float32)).astype(np.float32)


# revision 26
# speedup vs baseline: 1.0911x; 1.0058x over previous
"""M3GNet interaction kernel for 8 Trainium2 NeuronCores.

Sharding: edges (640000) and triplets (1000000) are split 8 ways
(graph/data parallel, per the sharding hint); weight matrices are
replicated. Each core runs the dense per-edge radial MLP and the
per-triplet angular MLP (first layer + shifted-softplus) on device;
per-node segment sums are combined after gathering the shards.

Device-kernel layout tricks:
 - Pairs of 512-element slices are stacked on partition halves
   (block-diagonal weights), so every matmul / activation runs with
   all 128 partitions instead of 64.
 - Matmuls run in bf16 (edge path) / fp32r (triplet path, N=512 so
   full rate) instead of fp32 (which costs 4 cycles/column).
 - softplus = Ln(1 + Exp(x)) with the +1 folded into the Ln bias; both
   funcs live in one activation table (natural_log_exp_and_others) so
   no ACT_TABLE_LOAD thrash.  The -log2 shift is folded into a bias
   column (edge path) / host-side count correction (triplet path).
"""
import numpy as np

import concourse.bacc as bacc
import concourse.bass as bass
import concourse.mybir as mybir
from concourse.tile import TileContext
from concourse import bass_utils
import concourse.hw_specs as hw_specs

N_NODES = 20000
N_EDGES = 640000
N_TRIP = 1000000
C = 128
E = 64
CUTOFF = 5.0
LOG2 = float(np.log(2.0))
NCORES = 8
EPC = N_EDGES // NCORES      # 80000 edges per core
TPC = N_TRIP // NCORES       # 125000 triplets per core

EPAD = 81920                 # edges padded: 80 pairs of (512+512)
TPAD = 125952                # triplets padded: 123 pairs
ECOLS = EPAD // 2            # 40960 packed columns (two edges/col)
TCOLS = TPAD // 2            # 62976 packed columns

ECH = 4096                   # edge packed-cols per chunk (10 chunks)
TCH = 8192                   # triplet packed-cols per chunk (8 chunks)

GAMMA = 1.0 / (2.0 * (CUTOFF / E) ** 2)

_CACHED = {}


def _patch_act_tables():
    """Restrict activation-table choice to the single table that holds
    Exp+Ln (+Square/Copy), so the compiler stops alternating table
    loads between Exp and Ln (which cost ~675us in the fp32 baseline).
    Table list order (= act_func_set_id) is preserved."""
    if _CACHED.get('act_patched'):
        return
    orig = hw_specs.get_activation_tables

    def patched(arch):
        return {k: (v if k == 'natural_log_exp_and_others' else set())
                for k, v in orig(arch).items()}

    bacc.get_activation_tables = patched
    _CACHED['act_patched'] = True


def _build():
    if 'nc' in _CACHED:
        return _CACHED['nc']
    _patch_act_tables()
    nc = bacc.Bacc('TRN2', target_bir_lowering=False, debug=False)
    f32 = mybir.dt.float32
    bf = mybir.dt.bfloat16
    f32r = mybir.dt.float32r

    rbe = nc.dram_tensor('rbe', [128, ECOLS], bf, kind='ExternalInput')
    tbf = nc.dram_tensor('tbf', [6, TCOLS], f32r, kind='ExternalInput')
    # wcat = [w1blk | w2dup] bf16; fcat = [w3blk_padded | corr] fp32(r)
    wcat = nc.dram_tensor('wcat', [128, 256], bf, kind='ExternalInput')
    fcat = nc.dram_tensor('fcat', [128, 129], f32r, kind='ExternalInput')

    moT = nc.dram_tensor('moT', [C, EPAD], bf, kind='ExternalOutput')
    uT = nc.dram_tensor('uT', [128, TCOLS], bf, kind='ExternalOutput')

    AF = mybir.ActivationFunctionType
    with TileContext(nc) as tc:
        with (
            tc.tile_pool(name='wpool', bufs=1) as wp,
            tc.tile_pool(name='rbe_in', bufs=3) as rin,
            tc.tile_pool(name='tbf_in', bufs=2) as tin,
            tc.tile_pool(name='expbuf', bufs=2) as eb,
            tc.tile_pool(name='spbuf', bufs=2) as sb,
            tc.tile_pool(name='outbuf', bufs=2) as ob,
            tc.tile_pool(name='psA', bufs=2, space='PSUM') as psA,
            tc.tile_pool(name='psB', bufs=2, space='PSUM') as psB,
        ):
            wct = wp.tile([128, 256], bf, tag='wc')
            nc.sync.dma_start(wct[:], wcat[:])
            fct = wp.tile([128, 129], f32r, tag='fc')
            nc.sync.dma_start(fct[:], fcat[:])
            w1t = wct[:, 0:128]
            w2t = wct[:, 128:256]
            w3t = fct[0:6, 0:128]
            corr_t = fct[:, 128:129].bitcast(f32)

            # ---- edge chunk: packed col j holds edges (1024p + q) [top]
            # and (1024p + 512 + q) [bottom] where j = 512p + q.
            def edge_chunk(c0):
                cw = min(ECH, ECOLS - c0)
                rt = rin.tile([128, ECH], bf, tag='rbe')
                nc.sync.dma_start(rt[:, :cw], rbe[:, c0:c0 + cw])
                et = eb.tile([128, ECH], bf, tag='e16')
                for q0 in range(0, cw, 1024):
                    qw = min(1024, cw - q0)
                    pp = psA.tile([128, 1024], f32, tag='pp')
                    for s0 in range(0, qw, 512):
                        nc.tensor.matmul(pp[:, s0:s0 + 512], w1t[:],
                                         rt[:, q0 + s0:q0 + s0 + 512])
                    nc.scalar.activation(et[:, q0:q0 + qw], pp[:, :qw], AF.Exp)
                st = sb.tile([128, ECH], bf, tag='sp')
                nc.scalar.activation(st[:, :cw], et[:, :cw], AF.Ln, bias=1.0)
                mt = ob.tile([128, 2 * ECH], bf, tag='mo')
                for p in range(cw // 512):
                    j0 = p * 512
                    pq = psB.tile([128, 1024], f32, tag='pq')
                    nc.tensor.matmul(pq[:, :512], w2t[0:64, :],
                                     st[0:64, j0:j0 + 512])
                    nc.tensor.matmul(pq[:, 512:], w2t[64:128, :],
                                     st[64:128, j0:j0 + 512])
                    nc.vector.tensor_scalar(mt[:, 2 * j0:2 * j0 + 1024],
                                            pq[:], corr_t[:], None,
                                            mybir.AluOpType.add)
                    if p % 4 == 3:
                        # flush per 4 pairs (1 MB): smoother DMA overlap
                        f0 = (p - 3) * 1024
                        nc.sync.dma_start(
                            moT[:, 2 * c0 + f0:2 * c0 + f0 + 4096],
                            mt[:, f0:f0 + 4096])

            # ---- triplet chunk: u = ln(1 + exp(tbf @ W3b1)) ----
            def trip_chunk(c0):
                cw = min(TCH, TCOLS - c0)
                tt = tin.tile([6, TCH], f32r, tag='tb')
                nc.sync.dma_start(tt[:, :cw], tbf[:, c0:c0 + cw])
                ut = ob.tile([128, TCH], bf, tag='u16')
                for g0 in range(0, cw, 4096):
                    gw = min(4096, cw - g0)
                    e3 = eb.tile([128, 4096], bf, tag='e3')
                    for q0 in range(g0, g0 + gw, 1024):
                        qw = min(1024, g0 + gw - q0)
                        # psB is idle during the triplet phase: alternate
                        # pools so 4 PSUM tiles pipeline and PE stays warm
                        if (q0 // 1024) % 2 == 0:
                            pp = psA.tile([128, 1024], f32, tag='pp')
                        else:
                            pp = psB.tile([128, 1024], f32, tag='pq')
                        for s0 in range(0, qw, 512):
                            nc.tensor.matmul(
                                pp[:, s0:s0 + 512],
                                w3t[:], tt[:, q0 + s0:q0 + s0 + 512])
                        nc.scalar.activation(e3[:, q0 - g0:q0 - g0 + qw],
                                             pp[:, :qw], AF.Exp)
                    nc.scalar.activation(ut[:, g0:g0 + gw], e3[:, :gw],
                                         AF.Ln, bias=1.0)
                    # per-half DMA keeps the kernel-tail transfer small
                    nc.sync.dma_start(uT[:, c0 + g0:c0 + g0 + gw],
                                      ut[:, g0:g0 + gw])

            for c0 in range(0, ECOLS, ECH):
                edge_chunk(c0)
            for c0 in range(0, TCOLS, TCH):
                trip_chunk(c0)

    nc.compile()
    _CACHED['nc'] = nc
    return nc


def _segsum(vals, idx, nseg):
    """f64-accurate segment sum via sort + cumsum (duplicate-safe)."""
    order = np.argsort(idx, kind='stable')
    sidx = idx[order]
    cs = np.cumsum(vals[order].astype(np.float64), axis=0)
    csz = np.vstack([np.zeros((1, vals.shape[1])), cs])
    starts = np.searchsorted(sidx, np.arange(nseg), side='left')
    ends = np.searchsorted(sidx, np.arange(nseg), side='right')
    return (csz[ends] - csz[starts]).astype(np.float32)


def _pack_pairs_edges(x):
    """[EPAD, 64] -> [128, ECOLS]: col 512p+q holds rows 1024p+q (top
    64 partitions) and 1024p+512+q (bottom 64)."""
    return np.ascontiguousarray(
        x.reshape(-1, 2, 512, 64).transpose(1, 3, 0, 2).reshape(128, -1))


def _pack_pairs_tbf(x):
    """[3, TPAD] -> [6, TCOLS]: col 512p+q holds triplet 1024p+q
    (rows 0-2) and 1024p+512+q (rows 3-5)."""
    return np.ascontiguousarray(
        x.reshape(3, -1, 2, 512).transpose(2, 0, 1, 3).reshape(6, -1))


def _unpack_pairs_u(uT):
    """[128, TCOLS] -> [TPAD, 64] (inverse of the pair packing)."""
    return uT.reshape(2, 64, -1, 512).transpose(2, 0, 3, 1).reshape(-1, 64)


def kernel(features, neighbour_distances, neighbour_list, triplet_idxs,
           angles, r_ij, r_ik, W_pre, W2b1, W2b2, W3b1, W3b2, W_post):
    nc = _build()
    bf16 = mybir.dt.np(mybir.dt.bfloat16)

    d = np.asarray(neighbour_distances, np.float32)
    env = (0.5 * (1.0 + np.cos(np.pi * d / CUTOFF))
           * (d < CUTOFF)).astype(np.float32)
    centers = np.linspace(0.0, CUTOFF, E, dtype=np.float32)
    rbe_full = (np.exp(-GAMMA * (d[:, None] - centers[None, :]) ** 2)
                * env[:, None]).astype(np.float32)          # [Ne, 64]
    tbf_full = np.stack([np.asarray(r_ij, np.float32),
                         np.asarray(r_ik, np.float32),
                         np.cos(np.asarray(angles, np.float32))], axis=0)

    W2b1 = np.asarray(W2b1, np.float32)
    W2b2 = np.asarray(W2b2, np.float32)
    W3b1 = np.asarray(W3b1, np.float32)
    wcat = np.zeros((128, 256), np.float32)
    wcat[:64, :64] = W2b1            # w1blk block-diagonal
    wcat[64:, 64:128] = W2b1
    wcat[:64, 128:] = W2b2           # w2dup (both halves)
    wcat[64:, 128:] = W2b2
    fcat = np.zeros((128, 129), np.float32)
    fcat[0:3, 0:64] = W3b1           # w3blk block-diagonal
    fcat[3:6, 64:128] = W3b1
    fcat[:, 128] = -LOG2 * W2b2.sum(axis=0)   # corr column

    shared = {
        'wcat': wcat.astype(bf16),
        'fcat': np.ascontiguousarray(fcat),
    }
    in_maps = []
    for k in range(NCORES):
        ec = np.zeros((EPAD, E), np.float32)
        ec[:EPC] = rbe_full[k * EPC:(k + 1) * EPC]
        tc_ = np.zeros((3, TPAD), np.float32)
        tc_[:, :TPC] = tbf_full[:, k * TPC:(k + 1) * TPC]
        in_maps.append(dict(shared,
                            rbe=_pack_pairs_edges(ec).astype(bf16),
                            tbf=_pack_pairs_tbf(tc_)))

    res = bass_utils.run_bass_kernel_spmd(nc, in_maps,
                                          core_ids=list(range(NCORES)))
    kernel.last_results = res

    m = np.concatenate(
        [r['moT'][:, :EPC].astype(np.float32).T for r in res.results],
        axis=0)                                            # [Ne, C]
    u = np.concatenate(
        [_unpack_pairs_u(r['uT'].astype(np.float32))[:TPC]
         for r in res.results], axis=0)                    # [Nt, E]

    h = np.asarray(features, np.float32) @ np.asarray(W_pre, np.float32)
    nl0 = np.asarray(neighbour_list)[0]
    nl1 = np.asarray(neighbour_list)[1]
    t1 = np.asarray(triplet_idxs)[:, 1]

    two_body = h[nl1] * m
    agg = _segsum(two_body, nl0, N_NODES)

    U3 = _segsum(u, t1, N_NODES)
    U3 -= LOG2 * np.bincount(t1, minlength=N_NODES)[:, None]
    em = h[:N_NODES] * (U3 @ np.asarray(W3b2, np.float32))
    agg += _segsum(em, nl0[:N_NODES], N_NODES)

    return (agg @ np.asarray(W_post, np.float32)).astype(np.float32)


# revision 28
# speedup vs baseline: 1.1015x; 1.0095x over previous
"""M3GNet interaction kernel for 8 Trainium2 NeuronCores.

Sharding: edges (640000) and triplets (1000000) are split 8 ways
(graph/data parallel, per the sharding hint); weight matrices are
replicated. Each core runs the dense per-edge radial MLP and the
per-triplet angular MLP (first layer + shifted-softplus) on device;
per-node segment sums are combined after gathering the shards.

Device-kernel layout tricks:
 - Pairs of 512-element slices are stacked on partition halves
   (block-diagonal weights), so every matmul / activation runs with
   all 128 partitions instead of 64.
 - Matmuls run in bf16 (edge path) / fp32r (triplet path, N=512 so
   full rate) instead of fp32 (which costs 4 cycles/column).
 - softplus = Ln(1 + Exp(x)) with the +1 folded into the Ln bias; both
   funcs live in one activation table (natural_log_exp_and_others) so
   no ACT_TABLE_LOAD thrash.  The -log2 shift is folded into a bias
   column (edge path) / host-side count correction (triplet path).
"""
import numpy as np

import concourse.bacc as bacc
import concourse.bass as bass
import concourse.mybir as mybir
from concourse.tile import TileContext
from concourse import bass_utils
import concourse.hw_specs as hw_specs

N_NODES = 20000
N_EDGES = 640000
N_TRIP = 1000000
C = 128
E = 64
CUTOFF = 5.0
LOG2 = float(np.log(2.0))
NCORES = 8
EPC = N_EDGES // NCORES      # 80000 edges per core
TPC = N_TRIP // NCORES       # 125000 triplets per core

EPAD = 81920                 # edges padded: 80 pairs of (512+512)
TPAD = 125952                # triplets padded: 123 pairs
ECOLS = EPAD // 2            # 40960 packed columns (two edges/col)
TCOLS = TPAD // 2            # 62976 packed columns

ECH = 4096                   # edge packed-cols per chunk (10 chunks)
TCH = 8192                   # triplet packed-cols per chunk (8 chunks)

GAMMA = 1.0 / (2.0 * (CUTOFF / E) ** 2)

_CACHED = {}


def _patch_act_tables():
    """Restrict activation-table choice to the single table that holds
    Exp+Ln (+Square/Copy), so the compiler stops alternating table
    loads between Exp and Ln (which cost ~675us in the fp32 baseline).
    Table list order (= act_func_set_id) is preserved."""
    if _CACHED.get('act_patched'):
        return
    orig = hw_specs.get_activation_tables

    def patched(arch):
        return {k: (v if k == 'natural_log_exp_and_others' else set())
                for k, v in orig(arch).items()}

    bacc.get_activation_tables = patched
    _CACHED['act_patched'] = True


def _build():
    if 'nc' in _CACHED:
        return _CACHED['nc']
    _patch_act_tables()
    nc = bacc.Bacc('TRN2', target_bir_lowering=False, debug=False)
    f32 = mybir.dt.float32
    bf = mybir.dt.bfloat16
    f32r = mybir.dt.float32r

    rbe = nc.dram_tensor('rbe', [128, ECOLS], bf, kind='ExternalInput')
    tbf = nc.dram_tensor('tbf', [6, TCOLS], f32r, kind='ExternalInput')
    # wcat = [w1blk | w2dup] bf16; fcat = [w3blk_padded | corr] fp32(r)
    wcat = nc.dram_tensor('wcat', [128, 256], bf, kind='ExternalInput')
    fcat = nc.dram_tensor('fcat', [128, 129], f32r, kind='ExternalInput')

    moT = nc.dram_tensor('moT', [C, EPAD], bf, kind='ExternalOutput')
    uT = nc.dram_tensor('uT', [128, TCOLS], bf, kind='ExternalOutput')

    AF = mybir.ActivationFunctionType
    with TileContext(nc) as tc:
        with (
            tc.tile_pool(name='wpool', bufs=1) as wp,
            tc.tile_pool(name='rbe_in', bufs=3) as rin,
            tc.tile_pool(name='tbf_in', bufs=2) as tin,
            tc.tile_pool(name='expbuf', bufs=2) as eb,
            tc.tile_pool(name='spbuf', bufs=2) as sb,
            tc.tile_pool(name='outbuf', bufs=2) as ob,
            tc.tile_pool(name='psA', bufs=2, space='PSUM') as psA,
            tc.tile_pool(name='psB', bufs=2, space='PSUM') as psB,
        ):
            wct = wp.tile([128, 256], bf, tag='wc')
            nc.sync.dma_start(wct[:], wcat[:])
            fct = wp.tile([128, 129], f32r, tag='fc')
            nc.sync.dma_start(fct[:], fcat[:])
            w1t = wct[:, 0:128]
            w2t = wct[:, 128:256]
            w3t = fct[0:6, 0:128]
            corr_t = fct[:, 128:129].bitcast(f32)

            # ---- edge chunk: packed col j holds edges (1024p + q) [top]
            # and (1024p + 512 + q) [bottom] where j = 512p + q.
            def edge_chunk(c0):
                cw = min(ECH, ECOLS - c0)
                rt = rin.tile([128, ECH], bf, tag='rbe')
                # inputs ride the (idle) gpsimd SWDGE queue so prefetch is
                # never FIFO-blocked behind output drains on the sync queue
                nc.gpsimd.dma_start(rt[:, :cw], rbe[:, c0:c0 + cw])
                et = eb.tile([128, ECH], bf, tag='e16')
                for q0 in range(0, cw, 1024):
                    qw = min(1024, cw - q0)
                    pp = psA.tile([128, 1024], f32, tag='pp')
                    for s0 in range(0, qw, 512):
                        nc.tensor.matmul(pp[:, s0:s0 + 512], w1t[:],
                                         rt[:, q0 + s0:q0 + s0 + 512])
                    nc.scalar.activation(et[:, q0:q0 + qw], pp[:, :qw], AF.Exp)
                st = sb.tile([128, ECH], bf, tag='sp')
                nc.scalar.activation(st[:, :cw], et[:, :cw], AF.Ln, bias=1.0)
                mt = ob.tile([128, 2 * ECH], bf, tag='mo')
                for p in range(cw // 512):
                    j0 = p * 512
                    pq = psB.tile([128, 1024], f32, tag='pq')
                    nc.tensor.matmul(pq[:, :512], w2t[0:64, :],
                                     st[0:64, j0:j0 + 512])
                    nc.tensor.matmul(pq[:, 512:], w2t[64:128, :],
                                     st[64:128, j0:j0 + 512])
                    nc.vector.tensor_scalar(mt[:, 2 * j0:2 * j0 + 1024],
                                            pq[:], corr_t[:], None,
                                            mybir.AluOpType.add)
                    if p % 4 == 3:
                        # flush per 4 pairs (1 MB): smoother DMA overlap
                        f0 = (p - 3) * 1024
                        nc.sync.dma_start(
                            moT[:, 2 * c0 + f0:2 * c0 + f0 + 4096],
                            mt[:, f0:f0 + 4096])

            # ---- triplet chunk: u = ln(1 + exp(tbf @ W3b1)) ----
            def trip_chunk(c0):
                cw = min(TCH, TCOLS - c0)
                tt = tin.tile([6, TCH], f32r, tag='tb')
                nc.gpsimd.dma_start(tt[:, :cw], tbf[:, c0:c0 + cw])
                ut = ob.tile([128, TCH], bf, tag='u16')
                for g0 in range(0, cw, 4096):
                    gw = min(4096, cw - g0)
                    e3 = eb.tile([128, 4096], bf, tag='e3')
                    for q0 in range(g0, g0 + gw, 1024):
                        qw = min(1024, g0 + gw - q0)
                        # psB is idle during the triplet phase: alternate
                        # pools so 4 PSUM tiles pipeline and PE stays warm
                        if (q0 // 1024) % 2 == 0:
                            pp = psA.tile([128, 1024], f32, tag='pp')
                        else:
                            pp = psB.tile([128, 1024], f32, tag='pq')
                        for s0 in range(0, qw, 512):
                            nc.tensor.matmul(
                                pp[:, s0:s0 + 512],
                                w3t[:], tt[:, q0 + s0:q0 + s0 + 512])
                        nc.scalar.activation(e3[:, q0 - g0:q0 - g0 + qw],
                                             pp[:, :qw], AF.Exp)
                    nc.scalar.activation(ut[:, g0:g0 + gw], e3[:, :gw],
                                         AF.Ln, bias=1.0)
                    # per-half DMA keeps the kernel-tail transfer small
                    nc.sync.dma_start(uT[:, c0 + g0:c0 + g0 + gw],
                                      ut[:, g0:g0 + gw])

            for c0 in range(0, ECOLS, ECH):
                edge_chunk(c0)
            for c0 in range(0, TCOLS, TCH):
                trip_chunk(c0)

    nc.compile()
    _CACHED['nc'] = nc
    return nc


def _segsum(vals, idx, nseg):
    """f64-accurate segment sum via sort + cumsum (duplicate-safe)."""
    order = np.argsort(idx, kind='stable')
    sidx = idx[order]
    cs = np.cumsum(vals[order].astype(np.float64), axis=0)
    csz = np.vstack([np.zeros((1, vals.shape[1])), cs])
    starts = np.searchsorted(sidx, np.arange(nseg), side='left')
    ends = np.searchsorted(sidx, np.arange(nseg), side='right')
    return (csz[ends] - csz[starts]).astype(np.float32)


def _pack_pairs_edges(x):
    """[EPAD, 64] -> [128, ECOLS]: col 512p+q holds rows 1024p+q (top
    64 partitions) and 1024p+512+q (bottom 64)."""
    return np.ascontiguousarray(
        x.reshape(-1, 2, 512, 64).transpose(1, 3, 0, 2).reshape(128, -1))


def _pack_pairs_tbf(x):
    """[3, TPAD] -> [6, TCOLS]: col 512p+q holds triplet 1024p+q
    (rows 0-2) and 1024p+512+q (rows 3-5)."""
    return np.ascontiguousarray(
        x.reshape(3, -1, 2, 512).transpose(2, 0, 1, 3).reshape(6, -1))


def _unpack_pairs_u(uT):
    """[128, TCOLS] -> [TPAD, 64] (inverse of the pair packing)."""
    return uT.reshape(2, 64, -1, 512).transpose(2, 0, 3, 1).reshape(-1, 64)


def kernel(features, neighbour_distances, neighbour_list, triplet_idxs,
           angles, r_ij, r_ik, W_pre, W2b1, W2b2, W3b1, W3b2, W_post):
    nc = _build()
    bf16 = mybir.dt.np(mybir.dt.bfloat16)

    d = np.asarray(neighbour_distances, np.float32)
    env = (0.5 * (1.0 + np.cos(np.pi * d / CUTOFF))
           * (d < CUTOFF)).astype(np.float32)
    centers = np.linspace(0.0, CUTOFF, E, dtype=np.float32)
    rbe_full = (np.exp(-GAMMA * (d[:, None] - centers[None, :]) ** 2)
                * env[:, None]).astype(np.float32)          # [Ne, 64]
    tbf_full = np.stack([np.asarray(r_ij, np.float32),
                         np.asarray(r_ik, np.float32),
                         np.cos(np.asarray(angles, np.float32))], axis=0)

    W2b1 = np.asarray(W2b1, np.float32)
    W2b2 = np.asarray(W2b2, np.float32)
    W3b1 = np.asarray(W3b1, np.float32)
    wcat = np.zeros((128, 256), np.float32)
    wcat[:64, :64] = W2b1            # w1blk block-diagonal
    wcat[64:, 64:128] = W2b1
    wcat[:64, 128:] = W2b2           # w2dup (both halves)
    wcat[64:, 128:] = W2b2
    fcat = np.zeros((128, 129), np.float32)
    fcat[0:3, 0:64] = W3b1           # w3blk block-diagonal
    fcat[3:6, 64:128] = W3b1
    fcat[:, 128] = -LOG2 * W2b2.sum(axis=0)   # corr column

    shared = {
        'wcat': wcat.astype(bf16),
        'fcat': np.ascontiguousarray(fcat),
    }
    in_maps = []
    for k in range(NCORES):
        ec = np.zeros((EPAD, E), np.float32)
        ec[:EPC] = rbe_full[k * EPC:(k + 1) * EPC]
        tc_ = np.zeros((3, TPAD), np.float32)
        tc_[:, :TPC] = tbf_full[:, k * TPC:(k + 1) * TPC]
        in_maps.append(dict(shared,
                            rbe=_pack_pairs_edges(ec).astype(bf16),
                            tbf=_pack_pairs_tbf(tc_)))

    res = bass_utils.run_bass_kernel_spmd(nc, in_maps,
                                          core_ids=list(range(NCORES)))
    kernel.last_results = res

    m = np.concatenate(
        [r['moT'][:, :EPC].astype(np.float32).T for r in res.results],
        axis=0)                                            # [Ne, C]
    u = np.concatenate(
        [_unpack_pairs_u(r['uT'].astype(np.float32))[:TPC]
         for r in res.results], axis=0)                    # [Nt, E]

    h = np.asarray(features, np.float32) @ np.asarray(W_pre, np.float32)
    nl0 = np.asarray(neighbour_list)[0]
    nl1 = np.asarray(neighbour_list)[1]
    t1 = np.asarray(triplet_idxs)[:, 1]

    two_body = h[nl1] * m
    agg = _segsum(two_body, nl0, N_NODES)

    U3 = _segsum(u, t1, N_NODES)
    U3 -= LOG2 * np.bincount(t1, minlength=N_NODES)[:, None]
    em = h[:N_NODES] * (U3 @ np.asarray(W3b2, np.float32))
    agg += _segsum(em, nl0[:N_NODES], N_NODES)

    return (agg @ np.asarray(W_post, np.float32)).astype(np.float32)


# revision 29
# speedup vs baseline: 1.1623x; 1.0551x over previous
"""M3GNet interaction kernel for 8 Trainium2 NeuronCores.

Sharding: edges (640000) and triplets (1000000) are split 8 ways
(graph/data parallel, per the sharding hint); weight matrices are
replicated. Each core runs the dense per-edge radial MLP and the
per-triplet angular MLP (first layer + shifted-softplus) on device;
per-node segment sums are combined after gathering the shards.

Device-kernel layout tricks:
 - Pairs of 512-element slices are stacked on partition halves
   (block-diagonal weights), so every matmul / activation runs with
   all 128 partitions instead of 64.
 - Matmuls run in bf16 (edge path) / fp32r (triplet path, N=512 so
   full rate) instead of fp32 (which costs 4 cycles/column).
 - softplus = Ln(1 + Exp(x)) with the +1 folded into the Ln bias; both
   funcs live in one activation table (natural_log_exp_and_others) so
   no ACT_TABLE_LOAD thrash.  The -log2 shift is folded into a bias
   column (edge path) / host-side count correction (triplet path).
"""
import numpy as np

import concourse.bacc as bacc
import concourse.bass as bass
import concourse.mybir as mybir
from concourse.tile import TileContext
from concourse import bass_utils
import concourse.hw_specs as hw_specs

N_NODES = 20000
N_EDGES = 640000
N_TRIP = 1000000
C = 128
E = 64
CUTOFF = 5.0
LOG2 = float(np.log(2.0))
NCORES = 8
EPC = N_EDGES // NCORES      # 80000 edges per core
TPC = N_TRIP // NCORES       # 125000 triplets per core

EPAD = 81920                 # edges padded: 80 pairs of (512+512)
TPAD = 125952                # triplets padded: 123 pairs
ECOLS = EPAD // 2            # 40960 packed columns (two edges/col)
TCOLS = TPAD // 2            # 62976 packed columns

ECH = 4096                   # edge packed-cols per chunk (10 chunks)
TCH = 8192                   # triplet packed-cols per chunk (8 chunks)

GAMMA = 1.0 / (2.0 * (CUTOFF / E) ** 2)

_CACHED = {}


def _patch_act_tables():
    """Restrict activation-table choice to the single table that holds
    Exp+Ln (+Square/Copy), so the compiler stops alternating table
    loads between Exp and Ln (which cost ~675us in the fp32 baseline).
    Table list order (= act_func_set_id) is preserved."""
    if _CACHED.get('act_patched'):
        return
    orig = hw_specs.get_activation_tables

    def patched(arch):
        return {k: (v if k == 'natural_log_exp_and_others' else set())
                for k, v in orig(arch).items()}

    bacc.get_activation_tables = patched
    _CACHED['act_patched'] = True


def _build():
    if 'nc' in _CACHED:
        return _CACHED['nc']
    _patch_act_tables()
    nc = bacc.Bacc('TRN2', target_bir_lowering=False, debug=False)
    f32 = mybir.dt.float32
    bf = mybir.dt.bfloat16
    f32r = mybir.dt.float32r

    rbe = nc.dram_tensor('rbe', [128, ECOLS], bf, kind='ExternalInput')
    tbf = nc.dram_tensor('tbf', [6, TCOLS], f32r, kind='ExternalInput')
    # wcat = [w1blk | w2dup] bf16; fcat = [w3blk_padded | corr] fp32(r)
    wcat = nc.dram_tensor('wcat', [128, 256], bf, kind='ExternalInput')
    fcat = nc.dram_tensor('fcat', [128, 129], f32r, kind='ExternalInput')

    moT = nc.dram_tensor('moT', [C, EPAD], bf, kind='ExternalOutput')
    uT = nc.dram_tensor('uT', [128, TCOLS], bf, kind='ExternalOutput')

    AF = mybir.ActivationFunctionType
    with TileContext(nc) as tc:
        with (
            tc.tile_pool(name='wpool', bufs=1) as wp,
            tc.tile_pool(name='rbe_in', bufs=3) as rin,
            tc.tile_pool(name='tbf_in', bufs=2) as tin,
            tc.tile_pool(name='expbuf', bufs=2) as eb,
            tc.tile_pool(name='spbuf', bufs=2) as sb,
            tc.tile_pool(name='outbuf', bufs=2) as ob,
            tc.tile_pool(name='psA', bufs=2, space='PSUM') as psA,
            tc.tile_pool(name='psB', bufs=2, space='PSUM') as psB,
        ):
            wct = wp.tile([128, 256], bf, tag='wc')
            nc.sync.dma_start(wct[:], wcat[:])
            fct = wp.tile([128, 129], f32r, tag='fc')
            nc.sync.dma_start(fct[:], fcat[:])
            w1t = wct[:, 0:128]
            w2t = wct[:, 128:256]
            w3t = fct[0:6, 0:128]
            corr_t = fct[:, 128:129].bitcast(f32)

            # ---- edge chunk: packed col j holds edges (1024p + q) [top]
            # and (1024p + 512 + q) [bottom] where j = 512p + q.
            def edge_chunk(c0):
                cw = min(ECH, ECOLS - c0)
                rt = rin.tile([128, ECH], bf, tag='rbe')
                # inputs ride the (idle) gpsimd SWDGE queue so prefetch is
                # never FIFO-blocked behind output drains on the sync queue
                nc.gpsimd.dma_start(rt[:, :cw], rbe[:, c0:c0 + cw])
                et = eb.tile([128, ECH], bf, tag='e16')
                for q0 in range(0, cw, 1024):
                    qw = min(1024, cw - q0)
                    pp = psA.tile([128, 1024], f32, tag='pp')
                    for s0 in range(0, qw, 512):
                        nc.tensor.matmul(pp[:, s0:s0 + 512], w1t[:],
                                         rt[:, q0 + s0:q0 + s0 + 512])
                    nc.scalar.activation(et[:, q0:q0 + qw], pp[:, :qw], AF.Exp)
                st = sb.tile([128, ECH], bf, tag='sp')
                nc.scalar.activation(st[:, :cw], et[:, :cw], AF.Ln, bias=1.0)
                mt = ob.tile([128, 2 * ECH], bf, tag='mo')
                for p in range(cw // 512):
                    j0 = p * 512
                    pq = psB.tile([128, 1024], f32, tag='pq')
                    nc.tensor.matmul(pq[:, :512], w2t[0:64, :],
                                     st[0:64, j0:j0 + 512])
                    nc.tensor.matmul(pq[:, 512:], w2t[64:128, :],
                                     st[64:128, j0:j0 + 512])
                    nc.vector.tensor_scalar(mt[:, 2 * j0:2 * j0 + 1024],
                                            pq[:], corr_t[:], None,
                                            mybir.AluOpType.add)
                    if p % 4 == 3:
                        # flush per 4 pairs (1 MB): smoother DMA overlap
                        f0 = (p - 3) * 1024
                        nc.sync.dma_start(
                            moT[:, 2 * c0 + f0:2 * c0 + f0 + 4096],
                            mt[:, f0:f0 + 4096])

            # ---- triplet chunk: u = ln(1 + exp(tbf @ W3b1)) ----
            def trip_chunk(c0):
                cw = min(TCH, TCOLS - c0)
                tt = tin.tile([6, TCH], f32r, tag='tb')
                nc.gpsimd.dma_start(tt[:, :cw], tbf[:, c0:c0 + cw])
                ut = ob.tile([128, TCH], bf, tag='u16')
                for g0 in range(0, cw, 4096):
                    gw = min(4096, cw - g0)
                    e3 = eb.tile([128, 4096], bf, tag='e3')
                    for q0 in range(g0, g0 + gw, 1024):
                        qw = min(1024, g0 + gw - q0)
                        pp = psA.tile([128, 1024], f32, tag='pp')
                        for s0 in range(0, qw, 512):
                            nc.tensor.matmul(
                                pp[:, s0:s0 + 512],
                                w3t[:], tt[:, q0 + s0:q0 + s0 + 512])
                        nc.scalar.activation(e3[:, q0 - g0:q0 - g0 + qw],
                                             pp[:, :qw], AF.Exp)
                    nc.scalar.activation(ut[:, g0:g0 + gw], e3[:, :gw],
                                         AF.Ln, bias=1.0)
                    # per-half DMA keeps the kernel-tail transfer small
                    nc.sync.dma_start(uT[:, c0 + g0:c0 + g0 + gw],
                                      ut[:, g0:g0 + gw])

            for c0 in range(0, ECOLS, ECH):
                edge_chunk(c0)
            for c0 in range(0, TCOLS, TCH):
                trip_chunk(c0)

    nc.compile()
    _CACHED['nc'] = nc
    return nc


def _segsum(vals, idx, nseg):
    """f64-accurate segment sum via sort + cumsum (duplicate-safe)."""
    order = np.argsort(idx, kind='stable')
    sidx = idx[order]
    cs = np.cumsum(vals[order].astype(np.float64), axis=0)
    csz = np.vstack([np.zeros((1, vals.shape[1])), cs])
    starts = np.searchsorted(sidx, np.arange(nseg), side='left')
    ends = np.searchsorted(sidx, np.arange(nseg), side='right')
    return (csz[ends] - csz[starts]).astype(np.float32)


def _pack_pairs_edges(x):
    """[EPAD, 64] -> [128, ECOLS]: col 512p+q holds rows 1024p+q (top
    64 partitions) and 1024p+512+q (bottom 64)."""
    return np.ascontiguousarray(
        x.reshape(-1, 2, 512, 64).transpose(1, 3, 0, 2).reshape(128, -1))


def _pack_pairs_tbf(x):
    """[3, TPAD] -> [6, TCOLS]: col 512p+q holds triplet 1024p+q
    (rows 0-2) and 1024p+512+q (rows 3-5)."""
    return np.ascontiguousarray(
        x.reshape(3, -1, 2, 512).transpose(2, 0, 1, 3).reshape(6, -1))


def _unpack_pairs_u(uT):
    """[128, TCOLS] -> [TPAD, 64] (inverse of the pair packing)."""
    return uT.reshape(2, 64, -1, 512).transpose(2, 0, 3, 1).reshape(-1, 64)


def kernel(features, neighbour_distances, neighbour_list, triplet_idxs,
           angles, r_ij, r_ik, W_pre, W2b1, W2b2, W3b1, W3b2, W_post):
    nc = _build()
    bf16 = mybir.dt.np(mybir.dt.bfloat16)

    d = np.asarray(neighbour_distances, np.float32)
    env = (0.5 * (1.0 + np.cos(np.pi * d / CUTOFF))
           * (d < CUTOFF)).astype(np.float32)
    centers = np.linspace(0.0, CUTOFF, E, dtype=np.float32)
    rbe_full = (np.exp(-GAMMA * (d[:, None] - centers[None, :]) ** 2)
                * env[:, None]).astype(np.float32)          # [Ne, 64]
    tbf_full = np.stack([np.asarray(r_ij, np.float32),
                         np.asarray(r_ik, np.float32),
                         np.cos(np.asarray(angles, np.float32))], axis=0)

    W2b1 = np.asarray(W2b1, np.float32)
    W2b2 = np.asarray(W2b2, np.float32)
    W3b1 = np.asarray(W3b1, np.float32)
    wcat = np.zeros((128, 256), np.float32)
    wcat[:64, :64] = W2b1            # w1blk block-diagonal
    wcat[64:, 64:128] = W2b1
    wcat[:64, 128:] = W2b2           # w2dup (both halves)
    wcat[64:, 128:] = W2b2
    fcat = np.zeros((128, 129), np.float32)
    fcat[0:3, 0:64] = W3b1           # w3blk block-diagonal
    fcat[3:6, 64:128] = W3b1
    fcat[:, 128] = -LOG2 * W2b2.sum(axis=0)   # corr column

    shared = {
        'wcat': wcat.astype(bf16),
        'fcat': np.ascontiguousarray(fcat),
    }
    in_maps = []
    for k in range(NCORES):
        ec = np.zeros((EPAD, E), np.float32)
        ec[:EPC] = rbe_full[k * EPC:(k + 1) * EPC]
        tc_ = np.zeros((3, TPAD), np.float32)
        tc_[:, :TPC] = tbf_full[:, k * TPC:(k + 1) * TPC]
        in_maps.append(dict(shared,
                            rbe=_pack_pairs_edges(ec).astype(bf16),
                            tbf=_pack_pairs_tbf(tc_)))

    res = bass_utils.run_bass_kernel_spmd(nc, in_maps,
                                          core_ids=list(range(NCORES)))
    kernel.last_results = res

    m = np.concatenate(
        [r['moT'][:, :EPC].astype(np.float32).T for r in res.results],
        axis=0)                                            # [Ne, C]
    u = np.concatenate(
        [_unpack_pairs_u(r['uT'].astype(np.float32))[:TPC]
         for r in res.results], axis=0)                    # [Nt, E]

    h = np.asarray(features, np.float32) @ np.asarray(W_pre, np.float32)
    nl0 = np.asarray(neighbour_list)[0]
    nl1 = np.asarray(neighbour_list)[1]
    t1 = np.asarray(triplet_idxs)[:, 1]

    two_body = h[nl1] * m
    agg = _segsum(two_body, nl0, N_NODES)

    U3 = _segsum(u, t1, N_NODES)
    U3 -= LOG2 * np.bincount(t1, minlength=N_NODES)[:, None]
    em = h[:N_NODES] * (U3 @ np.asarray(W3b2, np.float32))
    agg += _segsum(em, nl0[:N_NODES], N_NODES)

    return (agg @ np.asarray(W_post, np.float32)).astype(np.float32)
